# revision 28
# baseline (speedup 1.0000x reference)
"""TRN2 Bass kernel for nn_KVGather: out[b,i,t] = kv[b, r_idx[b,i,t]] * r_weight[b,i,t].

Full shapes: r_idx/r_weight (32,49,4), kv (32,49,64,256) f32 -> out (32,49,4,64,256) f32.

Sharding: batch dim n=32 across 8 cores (4 batches/core), pure data parallel.

Device formulation (SPMD-static, memory-bound):
  The gather+scale is a matmul with a runtime selection matrix:
      out[j, :] = sum_r sel[r, j] * kv[r, :],   sel[r_j, j] = w_j (else 0).
  Per core, the 4 batches split into two 2-batch halves so the contraction
  (2*49 = 98 rows) fits a single TensorE pass (<=128). Per half:
  sel [98, 392] bf16 (host-built, runtime data), kv rows [98, 16384] bf16.
  TensorE computes 512-col f-slices into PSUM (1 bank each, 2 slices per
  2-bank psum tile, bufs=4); DVE/ACT alternate evacuating psum -> bf16
  staging (deep: 16 bufs, absorbs DMA backpressure so the PE never idles
  long enough for the HAM clock gate to re-throttle it to 1.2 GHz); HWDGE
  writes staging to DRAM (4 KB/partition descriptors, ~HBM line rate).
  The two 8-tile leftover blocks of both halves are merged into a single
  PE pass via column tiling (tile_position) and spread over the later
  passes to keep output density even. Program has no dynamic APs or
  register loads; indices/weights enter only through sel, so one compiled
  program serves all cores/inputs. HW exec ~106 us (baseline 362 us);
  ~90 us of that is the HBM roofline for 6.4 MB in + 25.7 MB out per core.

  bf16 keeps worst-case rel err ~1.2% (kv + sel rounding + psum->bf16
  round), inside the 2e-2 gate; the host upcasts the bf16 output to f32.
"""

import os
import sys

sys.path.insert(0, "/opt/trn_rl_repo")

import numpy as np
import ml_dtypes

N, P2, TOPK, HW_KV, C_KV = 32, 49, 4, 64, 256
NCORES = 8
NB = N // NCORES  # 4 batches per core
R2 = 2 * P2  # 98 rows per 2-batch half
F = HW_KV * C_KV  # 16384 elems per row
FS = 512  # f-slice: one PSUM bank of f32
FGRP = 4  # f-slices per psum tile / staging group
NFG = F // (FS * FGRP)  # 8 f-groups
TILES_HALF = 2 * P2 * TOPK  # 392 output tiles per half
TILES = 2 * TILES_HALF  # 784 per core
JBLOCKS = [(0, 128), (128, 128), (256, 128)]  # full blocks per half
RUNT0 = 384  # leftover 8 tiles per half; both halves merged in one PE pass
RUNTW = 8
KV0_CHUNKS = [2048, 2048, 4096, 8192]  # finer first chunks -> earlier first matmul
KV1_CHUNKS = [8192, 8192]  # interleaved into the h0 output stream

_compiled = None


def _build():
    import concourse.tile as tile
    from concourse import bacc, mybir

    nc = bacc.Bacc("TRN2", target_bir_lowering=False, debug=False)

    bf16 = mybir.dt.bfloat16
    f32 = mybir.dt.float32
    COPY = mybir.ActivationFunctionType.Copy

    kv_d = [
        nc.dram_tensor(f"kv{h}", [R2, F], bf16, kind="ExternalInput").ap()
        for h in (0, 1)
    ]
    sel_d = [
        nc.dram_tensor(f"sel{h}", [R2, TILES_HALF], bf16, kind="ExternalInput").ap()
        for h in (0, 1)
    ]
    out_d = nc.dram_tensor("out", [TILES, F], bf16, kind="ExternalOutput").ap()

    PSW = 2 * FS  # psum tile: 2 banks (1024 f32)
    STW = 2 * PSW  # stage tile: 2048 bf16 cols -> 4 KB/partition DMA descriptors

    with tile.TileContext(nc) as tc:
        with (
            tc.tile_pool(name="res", bufs=1) as res_pool,
            tc.tile_pool(name="stage", bufs=16) as stage_pool,
            tc.psum_pool(name="ps", bufs=4) as psum_pool,
        ):
            kv_sb = [
                res_pool.tile([R2, F], bf16, tag=f"kv{h}", name=f"kv_sb{h}")
                for h in (0, 1)
            ]
            sel_sb = [
                res_pool.tile(
                    [R2, TILES_HALF], bf16, tag=f"sel{h}", name=f"sel_sb{h}"
                )
                for h in (0, 1)
            ]

            nc.sync.dma_start(sel_sb[0][:], sel_d[0][:])
            c0 = 0
            for i, w in enumerate(KV0_CHUNKS):
                cs = slice(c0, c0 + w)
                nc.sync.dma_start(kv_sb[0][:, cs], kv_d[0][:, cs])
                c0 += w
                if i == 1:  # sel1 is only needed ~35us in (runt/h1 passes)
                    nc.sync.dma_start(sel_sb[1][:], sel_d[1][:])
            # front-load kv1 chunk 0: the stage pool (16 bufs a~0.5 MB) lets
            # producers run ahead while loads saturate HBM early
            cs = slice(0, KV1_CHUNKS[0])
            nc.sync.dma_start(kv_sb[1][:, cs], kv_d[1][:, cs])

            # out-DMA issue alternates between the two HWDGE rings (SP and
            # ACT sequencers) -- descriptor generation (~0.9us DIRECT2D per
            # DMA) on a single sequencer is nearly serialized against the
            # DMA drain itself; the ACT sequencer has idle slack since its
            # copies are dispatched ahead into the engine queue
            dma_flip = [0]

            def out_dma(dst, src):
                # sync takes 2/3 (it has more wait slack; ACT also runs copies)
                eng = nc.scalar if dma_flip[0] % 3 == 1 else nc.sync
                dma_flip[0] += 1
                eng.dma_start(dst, src)

            # ACT per-copy is cheaper than DVE but ACT also issues 1/3 of the
            # DMAs; balance total engine load at ~43% of copies on ACT
            copy_flip = [0]

            def psum_copy(dst, src):
                i = copy_flip[0]
                copy_flip[0] += 1
                if int((i + 1) * 0.43) > int(i * 0.43):
                    nc.scalar.activation(dst, src, COPY)
                else:
                    nc.vector.tensor_copy(dst, src)

            def do_stage(h, j0, jw, st):
                """One stage group: 2 psum tiles x 2 matmuls, 2 copies, 1 DMA."""
                stage = stage_pool.tile([128, STW], bf16, tag="st", name="stage")
                for k in range(2):
                    ps = psum_pool.tile([128, PSW], f32, tag="ps", name="ps")
                    for s in range(2):
                        fs = st * (STW // FS) + k * 2 + s
                        nc.tensor.matmul(
                            ps[:jw, s * FS : (s + 1) * FS],
                            sel_sb[h][:, j0 : j0 + jw],
                            kv_sb[h][:, fs * FS : (fs + 1) * FS],
                            start=True,
                            stop=True,
                        )
                    dst = stage[:jw, k * PSW : (k + 1) * PSW]
                    psum_copy(dst, ps[:jw])
                row0 = h * TILES_HALF + j0
                out_dma(
                    out_d[row0 : row0 + jw, st * STW : (st + 1) * STW],
                    stage[:jw],
                )

            def do_runt_stage(st):
                """Merged runt: both halves' last 8 tiles share each PE
                streaming pass via column tiling (h0 -> psum cols 0-31,
                h1 -> psum cols 32-63)."""
                stage = stage_pool.tile([128, STW], bf16, tag="st", name="stage_r")
                for k in range(2):
                    ps = psum_pool.tile([128, PSW], f32, tag="ps", name="ps_r")
                    for s in range(2):
                        fs = st * (STW // FS) + k * 2 + s
                        for h in (0, 1):
                            nc.tensor.matmul(
                                ps[
                                    32 * h : 32 * h + RUNTW,
                                    s * FS : (s + 1) * FS,
                                ],
                                sel_sb[h][:, RUNT0 : RUNT0 + RUNTW],
                                kv_sb[h][:, fs * FS : (fs + 1) * FS],
                                start=True,
                                stop=True,
                                tile_position=(0, 32 * h),
                            )
                    dst = stage[: 32 + RUNTW, k * PSW : (k + 1) * PSW]
                    psum_copy(dst, ps[: 32 + RUNTW])
                for h in (0, 1):
                    row0 = h * TILES_HALF + RUNT0
                    out_dma(
                        out_d[row0 : row0 + RUNTW, st * STW : (st + 1) * STW],
                        stage[32 * h : 32 * h + RUNTW],
                    )

            NST = F // STW  # 8 stage groups per block pass
            # 6 full passes; the 8 merged-runt stage groups are spread over
            # the last 4 passes (after st3 and st7) so output density stays
            # even and the DMA stream never starves
            passes = [(h, j0, jw) for h in (0, 1) for j0, jw in JBLOCKS]
            runt_next = 0
            for pi, (h, j0, jw) in enumerate(passes):
                for st in range(NST):
                    do_stage(h, j0, jw, st)
                    if pi >= 2 and st in (3, 7) and runt_next < NST:
                        do_runt_stage(runt_next)
                        runt_next += 1
                if pi == 0:  # second kv1 chunk right after the first pass
                    cs = slice(KV1_CHUNKS[0], F)
                    nc.sync.dma_start(kv_sb[1][:, cs], kv_d[1][:, cs])

    nc.compile()
    return nc


def _get_compiled():
    global _compiled
    if _compiled is None:
        _compiled = _build()
    return _compiled


def _enable_trace_hook():
    """Register the axon NTFF profile hook (missing antenv.axon_hooks shim)."""
    import types

    try:
        import antenv.axon_hooks  # noqa: F401

        return
    except ImportError:
        pass
    try:
        import antenv

        mod = types.ModuleType("antenv.axon_hooks")
        holder = {}
        mod.set_axon_ntff_profile_hook = lambda h: holder.__setitem__("h", h)
        mod.get_axon_ntff_profile_hook = lambda: holder.get("h")
        antenv.axon_hooks = mod
        sys.modules["antenv.axon_hooks"] = mod
        if "/root/.axon_site" not in sys.path:
            sys.path.insert(0, "/root/.axon_site")
        from trn_agent_boot.trn_boot import _ntff_profile_via_ctypes

        mod.set_axon_ntff_profile_hook(
            _ntff_profile_via_ctypes("/opt/axon/libaxon_pjrt.so")
        )

        import concourse.bass_utils as bu

        orig = bu.upload_artifacts

        def _safe_upload(tmpdir):
            try:
                return orig(tmpdir)
            except Exception:
                return tmpdir

        bu.upload_artifacts = _safe_upload
    except Exception as e:  # tracing is best-effort
        print(f"trace hook setup failed: {e}")


def kernel(r_idx, r_weight, kv):
    from concourse.bass_utils import run_bass_kernel_spmd

    r_idx = np.asarray(r_idx)
    r_weight = np.asarray(r_weight, dtype=np.float32)
    kv = np.asarray(kv, dtype=np.float32)
    assert r_idx.shape == (N, P2, TOPK) and kv.shape == (N, P2, HW_KV, C_KV)
    assert r_idx.min() >= 0 and r_idx.max() < P2

    nc = _get_compiled()

    bf16 = ml_dtypes.bfloat16
    jl = np.arange(TILES_HALF)
    in_maps = []
    for c in range(NCORES):
        b0 = c * NB
        kv_shard = kv[b0 : b0 + NB].reshape(2 * R2, F).astype(bf16)
        idx = np.asarray(r_idx[b0 : b0 + NB], dtype=np.int64).reshape(
            2, 2, P2, TOPK
        )  # (half, b2, i, t)
        w = r_weight[b0 : b0 + NB].reshape(2, 2, P2, TOPK)
        m = {}
        for h in (0, 1):
            m[f"kv{h}"] = np.ascontiguousarray(kv_shard[h * R2 : (h + 1) * R2])
            rloc = (np.arange(2)[:, None, None] * P2 + idx[h]).reshape(-1)
            sel = np.zeros((R2, TILES_HALF), dtype=np.float32)
            sel[rloc, jl] = w[h].reshape(-1)
            m[f"sel{h}"] = sel.astype(bf16)
        in_maps.append(m)

    trace = bool(int(os.environ.get("KV_TRACE", "0")))
    if trace:
        _enable_trace_hook()
    res = run_bass_kernel_spmd(nc, in_maps, list(range(NCORES)), trace=trace)

    if trace:
        kernel.last_exec_time_ns = res.exec_time_ns
        kernel.last_trace = (
            res.instructions_and_trace[1] if res.instructions_and_trace else None
        )

    out = np.empty((N, P2, TOPK, HW_KV, C_KV), dtype=np.float32)
    for c in range(NCORES):
        b0 = c * NB
        out[b0 : b0 + NB] = (
            np.asarray(res.results[c]["out"])
            .astype(np.float32)
            .reshape(NB, P2, TOPK, HW_KV, C_KV)
        )
    return out
